# revision 16
# baseline (speedup 1.0000x reference)
"""DBRX MoE FFN (top-2 of 8 experts, GLU) on 8 Trainium2 NeuronCores.

Strategy: expert-parallel with sparse token dispatch.
  - Host: router matmul + softmax + top-2 (0.02% of FLOPs), gather each
    expert's assigned tokens (~T*2/8 of T=4096), pad to a common capacity C,
    pre-transpose activations/weights into matmul-native layouts, cast to
    bf16 for the PE.
  - Device (core e = expert e): x1 = xT.T @ w1T, x2 = xT.T @ v1T with fp32
    PSUM accumulation; h = silu(x1) * x2; transpose h via the PE; y =
    sum_f hT.T @ w2 accumulated in PSUM; per-token gate multiply; DMA out.
  - Host: scatter-add per-expert outputs back to token order.

kernel(**inputs) takes the full unsharded inputs and returns
(out [B,S,D] fp32, router_weights [B,S,E] fp32) exactly like the reference.
"""

import os
import numpy as np
import ml_dtypes

B, S, D, E, F, TOPK = 2, 2048, 1024, 8, 2048, 2
T = B * S
P = 128
NCORES = 8
ND = D // P  # 8 contraction chunks for D
NF = F // P  # 16 contraction chunks for F
FQ = 512     # f-quarter (one PSUM bank of fp32)
WARMUP_OPS = int(os.environ.get("KERNEL_WARMUP_OPS", "40"))

_PROGRAM_CACHE = {}
LAST_RESULT = None  # stashed BassKernelResults for test harnesses


def _build_program(C):
    """Build + compile the per-core Bass program for token capacity C."""
    import concourse.mybir as mybir
    import concourse.tile as tile
    from concourse import bacc
    from concourse.masks import make_identity

    bf16 = mybir.dt.bfloat16
    f32 = mybir.dt.float32
    NT = C // P

    nc = bacc.Bacc("TRN2", target_bir_lowering=False, debug=False)
    xT_d = nc.dram_tensor("xT", [D, C], bf16, kind="ExternalInput").ap()
    w1_d = nc.dram_tensor("w1t", [D, F], bf16, kind="ExternalInput").ap()
    v1_d = nc.dram_tensor("v1t", [D, F], bf16, kind="ExternalInput").ap()
    w2_d = nc.dram_tensor("w2", [F, D], bf16, kind="ExternalInput").ap()
    gate_d = nc.dram_tensor("gate", [P, NT], f32, kind="ExternalInput").ap()
    y_d = nc.dram_tensor("y", [C, D], f32, kind="ExternalOutput").ap()

    with tile.TileContext(nc) as tc:
        with (
            tc.tile_pool(name="const", bufs=1) as const,
            tc.tile_pool(name="work", bufs=2) as work,
            tc.tile_pool(name="ps", bufs=2, space="PSUM") as ps,
            tc.tile_pool(name="ps_y", bufs=4, space="PSUM") as psy,
        ):
            ident = const.tile([P, P], bf16, tag="ident")
            make_identity(nc, ident)
            if WARMUP_OPS:
                # PE warmup: keep TensorE busy while input DMAs land so the
                # HAM clock gate opens (~3.4us busy) before real matmuls.
                for _ in range(WARMUP_OPS):
                    wtrp = psy.tile([P, P], bf16, tag="y")
                    nc.tensor.transpose(wtrp[:], ident[:], ident[:])

            gsb = const.tile([P, NT], f32, tag="gate")
            nc.sync.dma_start(gsb[:], gate_d[:])

            # DMA order is tuned so tile t=0 / q=0 inputs land first:
            # x columns for t0, then (w1,v1) 512-col slices in q order.
            xsb = []
            for d in range(ND):
                xsb.append(const.tile([P, C], bf16, tag=f"x_{d}", name=f"x_{d}"))
            for d in range(ND):
                nc.sync.dma_start(xsb[d][:, 0:P], xT_d[d * P:(d + 1) * P, 0:P])
            w1sb, v1sb = [], []
            for d in range(ND):
                w1sb.append(const.tile([P, F], bf16, tag=f"w1_{d}", name=f"w1_{d}"))
                v1sb.append(const.tile([P, F], bf16, tag=f"v1_{d}", name=f"v1_{d}"))
            w2sb = []
            for f in range(NF):
                w2sb.append(const.tile([P, D], bf16, tag=f"w2_{f}", name=f"w2_{f}"))
            for q in range(F // FQ):
                qsl = slice(q * FQ, (q + 1) * FQ)
                for d in range(ND):
                    nc.sync.dma_start(w1sb[d][:, qsl], w1_d[d * P:(d + 1) * P, qsl])
                for d in range(ND):
                    nc.sync.dma_start(v1sb[d][:, qsl], v1_d[d * P:(d + 1) * P, qsl])
                if q == 0:
                    for d in range(ND):
                        nc.sync.dma_start(
                            xsb[d][:, P:C], xT_d[d * P:(d + 1) * P, P:C]
                        )
                for j in range(FQ // P):
                    f = q * (FQ // P) + j
                    nc.sync.dma_start(w2sb[f][:], w2_d[f * P:(f + 1) * P, :])

            # Persistent per-tile fp32 output accumulators in SBUF.
            ysb = []
            for t in range(NT):
                ysb.append(const.tile([P, D], f32, tag=f"ysb_{t}", name=f"ysb_{t}"))

            NQ = F // FQ
            # q-outer / t-inner: each (w1,v1) 512-column tranche unlocks a
            # full pass over all token tiles (~30us of PE work per ~2MB of
            # weights), so the PE is never DMA-starved after the first tranche.
            for q in range(NQ):
                qsl = slice(q * FQ, (q + 1) * FQ)
                for t in range(NT):
                    tsl = slice(t * P, (t + 1) * P)
                    x1q = ps.tile([P, FQ], f32, tag="x1")
                    x2q = ps.tile([P, FQ], f32, tag="x2")
                    for d in range(ND):
                        nc.tensor.matmul(
                            x1q[:], xsb[d][:, tsl], w1sb[d][:, qsl],
                            start=(d == 0), stop=(d == ND - 1),
                        )
                    for d in range(ND):
                        nc.tensor.matmul(
                            x2q[:], xsb[d][:, tsl], v1sb[d][:, qsl],
                            start=(d == 0), stop=(d == ND - 1),
                        )
                    # silu(x1)*x2 = sigmoid(x1)*x1*x2 (CoreSim lacks a Silu LUT)
                    ssb = work.tile([P, FQ], f32, tag="s")
                    nc.scalar.activation(
                        ssb[:], x1q[:], mybir.ActivationFunctionType.Sigmoid
                    )
                    ssb2 = work.tile([P, FQ], f32, tag="s2")
                    nc.vector.tensor_mul(ssb2[:], ssb[:], x1q[:])
                    hq = work.tile([P, FQ], bf16, tag="h")
                    nc.vector.tensor_mul(hq[:], ssb2[:], x2q[:])
                    hT = work.tile([P, FQ // P, P], bf16, tag="hT")
                    y0 = psy.tile([P, FQ], f32, tag="y")
                    y1 = psy.tile([P, FQ], f32, tag="y")
                    for j in range(FQ // P):
                        f = q * (FQ // P) + j
                        # 128x128 transpose via the DMA XBAR on the Scalar
                        # HWDGE queue (keeps xbar-mode off the Sync copy queue
                        # and takes the transpose off the PE entirely).
                        nc.scalar.dma_start(
                            hT[:, j, :], hq[:, j * P:(j + 1) * P], transpose=True
                        )
                        nc.tensor.matmul(
                            y0[:], hT[:, j, :], w2sb[f][:, 0:FQ],
                            start=(j == 0), stop=(j == FQ // P - 1),
                        )
                        nc.tensor.matmul(
                            y1[:], hT[:, j, :], w2sb[f][:, FQ:2 * FQ],
                            start=(j == 0), stop=(j == FQ // P - 1),
                        )
                    # Epilogue split into d-halves so the final-tile gate and
                    # output DMA pipeline instead of serializing.
                    for half, yp in ((0, y0), (1, y1)):
                        hsl = slice(half * FQ, (half + 1) * FQ)
                        if q == 0:
                            nc.vector.tensor_copy(ysb[t][:, hsl], yp[:])
                        else:
                            nc.vector.tensor_add(ysb[t][:, hsl], ysb[t][:, hsl], yp[:])
                        if q == NQ - 1:
                            nc.vector.tensor_scalar_mul(
                                ysb[t][:, hsl], ysb[t][:, hsl], gsb[:, t:t + 1]
                            )
                            nc.sync.dma_start(y_d[tsl, hsl], ysb[t][:, hsl])

    nc.compile()
    return nc


def _get_program(C):
    if C not in _PROGRAM_CACHE:
        _PROGRAM_CACHE[C] = _build_program(C)
    return _PROGRAM_CACHE[C]


def _install_ntff_hook():
    """Best-effort registration of the axon NTFF profile hook (trace runs)."""
    import sys, types
    try:
        from antenv.axon_hooks import get_axon_ntff_profile_hook  # noqa: F401
        return
    except ImportError:
        pass
    mod = types.ModuleType("antenv.axon_hooks")
    mod._hook = None
    mod.set_axon_ntff_profile_hook = lambda h: setattr(mod, "_hook", h)
    mod.get_axon_ntff_profile_hook = lambda: mod._hook
    sys.modules["antenv.axon_hooks"] = mod
    try:
        from trn_agent_boot.trn_boot import _ntff_profile_via_ctypes
        mod._hook = _ntff_profile_via_ctypes("/opt/axon/libaxon_pjrt.so")
    except Exception:
        pass


def _route(x, router_kernel):
    """Replicates the reference router in fp32 numpy.

    Returns (weights [T,E], order [T,2] expert ids, twn [T,2] L1-normalized
    top-2 weights)."""
    logits = x @ router_kernel  # [T, E] fp32
    m = logits.max(axis=-1, keepdims=True)
    ex = np.exp(logits - m)
    weights = ex / ex.sum(axis=-1, keepdims=True)
    order = np.argsort(-weights, axis=-1, kind="stable")[:, :TOPK]
    tw = np.take_along_axis(weights, order, axis=-1)
    twn = tw / np.abs(tw).sum(axis=-1, keepdims=True)
    return weights.astype(np.float32), order, twn.astype(np.float32)


def kernel(hidden_states, router_kernel, w1, v1, w2):
    global LAST_RESULT
    from concourse.bass_utils import run_bass_kernel_spmd

    x = np.ascontiguousarray(np.asarray(hidden_states, dtype=np.float32)).reshape(T, D)
    rk = np.asarray(router_kernel, dtype=np.float32)
    w1 = np.asarray(w1, dtype=np.float32)
    v1 = np.asarray(v1, dtype=np.float32)
    w2 = np.asarray(w2, dtype=np.float32)

    weights, order, twn = _route(x, rk)

    idxs, gates = [], []
    for e in range(E):
        mask = (order[:, 0] == e) | (order[:, 1] == e)
        idx = np.nonzero(mask)[0]
        g = np.where(order[idx, 0] == e, twn[idx, 0], twn[idx, 1])
        idxs.append(idx)
        gates.append(g.astype(np.float32))
    Cmax = max(P, -(-max(len(i) for i in idxs) // P) * P)
    # SBUF-safe capacity cap; very skewed routings run in multiple chunks.
    CAP = int(os.environ.get("KERNEL_CHUNK_CAP", "1536"))
    C = min(Cmax, CAP)
    NT = C // P
    nchunks = -(-Cmax // C)

    nc = _get_program(C)

    trace = os.environ.get("KERNEL_TRACE") == "1"
    kwargs = {}
    if trace:
        _install_ntff_hook()
        kwargs["trace"] = True
        td = os.environ.get("KERNEL_TRACE_DIR")
        if td:
            kwargs["tmpdir"] = td

    x_bf = x.astype(ml_dtypes.bfloat16)
    wmaps = []
    for e in range(E):
        wmaps.append({
            "w1t": np.ascontiguousarray(w1[e].T.astype(ml_dtypes.bfloat16)),
            "v1t": np.ascontiguousarray(v1[e].T.astype(ml_dtypes.bfloat16)),
            "w2": np.ascontiguousarray(w2[e].astype(ml_dtypes.bfloat16)),
        })

    out = np.zeros((T, D), dtype=np.float32)
    for chunk in range(nchunks):
        c0 = chunk * C
        in_maps = []
        for e in range(E):
            idx = idxs[e][c0:c0 + C]
            n = len(idx)
            xg = np.zeros((C, D), dtype=ml_dtypes.bfloat16)
            xg[:n] = x_bf[idx]
            gate_pad = np.zeros(C, dtype=np.float32)
            gate_pad[:n] = gates[e][c0:c0 + C]
            in_maps.append({
                "xT": np.ascontiguousarray(xg.T),
                "gate": np.ascontiguousarray(gate_pad.reshape(NT, P).T),
                **wmaps[e],
            })
        res = run_bass_kernel_spmd(
            nc, in_maps, core_ids=list(range(NCORES)), **kwargs
        )
        LAST_RESULT = res
        for e in range(E):
            idx = idxs[e][c0:c0 + C]
            if len(idx):
                out[idx] += res.results[e]["y"][:len(idx)]
    return out.reshape(B, S, D), weights.reshape(B, S, E)


# revision 21
# speedup vs baseline: 1.8438x; 1.8438x over previous
"""DBRX MoE FFN (top-2 of 8 experts, GLU) on 8 Trainium2 NeuronCores.

Strategy: expert-parallel with sparse token dispatch.
  - Host: router matmul + softmax + top-2 (0.02% of FLOPs), gather each
    expert's assigned tokens (~T*2/8 of T=4096), pad to a common capacity C,
    pre-transpose activations/weights into matmul-native layouts, cast to
    bf16 for the PE.
  - Device (core e = expert e): x1 = xT.T @ w1T, x2 = xT.T @ v1T with fp32
    PSUM accumulation; h = silu(x1) * x2; transpose h via the PE; y =
    sum_f hT.T @ w2 accumulated in PSUM; per-token gate multiply; DMA out.
  - Host: scatter-add per-expert outputs back to token order.

kernel(**inputs) takes the full unsharded inputs and returns
(out [B,S,D] fp32, router_weights [B,S,E] fp32) exactly like the reference.
"""

import os
import numpy as np
import ml_dtypes

B, S, D, E, F, TOPK = 2, 2048, 1024, 8, 2048, 2
T = B * S
P = 128
NCORES = 8
ND = D // P  # 8 contraction chunks for D
NF = F // P  # 16 contraction chunks for F
FQ = 512     # f-quarter (one PSUM bank of fp32)
WARMUP_OPS = int(os.environ.get("KERNEL_WARMUP_OPS", "40"))
USE_SILU = os.environ.get("KERNEL_SILU", "1") == "1"

_PROGRAM_CACHE = {}
LAST_RESULT = None  # stashed BassKernelResults for test harnesses


def _build_program(C):
    """Build + compile the per-core Bass program for token capacity C."""
    import concourse.mybir as mybir
    import concourse.tile as tile
    from concourse import bacc
    from concourse.masks import make_identity

    bf16 = mybir.dt.bfloat16
    f32 = mybir.dt.float32
    NT = C // P

    nc = bacc.Bacc("TRN2", target_bir_lowering=False, debug=False)
    xT_d = nc.dram_tensor("xT", [D, C], bf16, kind="ExternalInput").ap()
    w1_d = nc.dram_tensor("w1t", [D, F], bf16, kind="ExternalInput").ap()
    v1_d = nc.dram_tensor("v1t", [D, F], bf16, kind="ExternalInput").ap()
    w2_d = nc.dram_tensor("w2", [F, D], bf16, kind="ExternalInput").ap()
    gate_d = nc.dram_tensor("gate", [P, NT], f32, kind="ExternalInput").ap()
    y_d = nc.dram_tensor("y", [C, D], f32, kind="ExternalOutput").ap()

    with tile.TileContext(nc) as tc:
        with (
            tc.tile_pool(name="const", bufs=1) as const,
            tc.tile_pool(name="work", bufs=2) as work,
            tc.tile_pool(name="ps", bufs=2, space="PSUM") as ps,
        ):
            ident = const.tile([P, P], bf16, tag="ident")
            make_identity(nc, ident)
            if WARMUP_OPS:
                # PE warmup: keep TensorE busy while input DMAs land so the
                # HAM clock gate opens (~3.4us busy) before real matmuls.
                for _ in range(WARMUP_OPS):
                    wtrp = ps.tile([P, P], bf16, tag="trp")
                    nc.tensor.transpose(wtrp[:], ident[:], ident[:])

            gsb = const.tile([P, NT], f32, tag="gate")
            nc.sync.dma_start(gsb[:], gate_d[:])

            # DMA order is tuned so tile t=0 / q=0 inputs land first:
            # x columns for t0, then (w1,v1) 512-col slices in q order.
            xsb = []
            for d in range(ND):
                xsb.append(const.tile([P, C], bf16, tag=f"x_{d}", name=f"x_{d}"))
            for d in range(ND):
                nc.sync.dma_start(xsb[d][:, 0:P], xT_d[d * P:(d + 1) * P, 0:P])
            w1sb, v1sb = [], []
            for d in range(ND):
                w1sb.append(const.tile([P, F], bf16, tag=f"w1_{d}", name=f"w1_{d}"))
                v1sb.append(const.tile([P, F], bf16, tag=f"v1_{d}", name=f"v1_{d}"))
            w2sb = []
            for f in range(NF):
                w2sb.append(const.tile([P, D], bf16, tag=f"w2_{f}", name=f"w2_{f}"))
            for q in range(F // FQ):
                qsl = slice(q * FQ, (q + 1) * FQ)
                for d in range(ND):
                    nc.sync.dma_start(w1sb[d][:, qsl], w1_d[d * P:(d + 1) * P, qsl])
                if q == 0:
                    # Remaining x columns land before v1 so phase-1 x1 work
                    # for every token tile is unlocked as early as possible.
                    for d in range(ND):
                        nc.sync.dma_start(
                            xsb[d][:, P:C], xT_d[d * P:(d + 1) * P, P:C]
                        )
                for d in range(ND):
                    nc.sync.dma_start(v1sb[d][:, qsl], v1_d[d * P:(d + 1) * P, qsl])
                for j in range(FQ // P):
                    f = q * (FQ // P) + j
                    nc.sync.dma_start(w2sb[f][:], w2_d[f * P:(f + 1) * P, :])

            # Persistent per-tile fp32 output accumulators in SBUF.
            ysb = []
            for t in range(NT):
                ysb.append(const.tile([P, D], f32, tag=f"ysb_{t}", name=f"ysb_{t}"))

            NQ = F // FQ
            # q-outer / t-inner: each (w1,v1) 512-column tranche unlocks a
            # full pass over all token tiles (~30us of PE work per ~2MB of
            # weights), so the PE is never DMA-starved after the first tranche.
            for q in range(NQ):
                qsl = slice(q * FQ, (q + 1) * FQ)
                for t in range(NT):
                    tsl = slice(t * P, (t + 1) * P)
                    x1q = ps.tile([P, FQ], f32, tag="x1")
                    x2q = ps.tile([P, FQ], f32, tag="x2")
                    for d in range(ND):
                        nc.tensor.matmul(
                            x1q[:], xsb[d][:, tsl], w1sb[d][:, qsl],
                            start=(d == 0), stop=(d == ND - 1),
                        )
                    for d in range(ND):
                        nc.tensor.matmul(
                            x2q[:], xsb[d][:, tsl], v1sb[d][:, qsl],
                            start=(d == 0), stop=(d == ND - 1),
                        )
                    hq = work.tile([P, FQ], bf16, tag="h")
                    ssb = work.tile([P, FQ], f32, tag="s")
                    if USE_SILU:
                        # HW Silu LUT on the Scalar engine (one DVE mul saved).
                        nc.scalar.activation(
                            ssb[:], x1q[:], mybir.ActivationFunctionType.Silu
                        )
                        nc.vector.tensor_mul(hq[:], ssb[:], x2q[:])
                    else:
                        # CoreSim path: silu(x1)*x2 = sigmoid(x1)*x1*x2
                        nc.scalar.activation(
                            ssb[:], x1q[:], mybir.ActivationFunctionType.Sigmoid
                        )
                        ssb2 = work.tile([P, FQ], f32, tag="s2")
                        nc.vector.tensor_mul(ssb2[:], ssb[:], x1q[:])
                        nc.vector.tensor_mul(hq[:], ssb2[:], x2q[:])
                    hT = work.tile([P, FQ // P, P], bf16, tag="hT")
                    y0 = ps.tile([P, FQ], f32, tag="y")
                    y1 = ps.tile([P, FQ], f32, tag="y")
                    for jj in range(FQ // P // 2):
                        # Two transposes share one PSUM tile -> one wider DVE
                        # copy back instead of two narrow ones.
                        trp = ps.tile([P, 2, P], bf16, tag="trp")
                        for j2 in range(2):
                            j = jj * 2 + j2
                            nc.tensor.transpose(
                                trp[:, j2, :], hq[:, j * P:(j + 1) * P], ident[:]
                            )
                        nc.vector.tensor_copy(hT[:, jj * 2:jj * 2 + 2, :], trp[:])
                        for j2 in range(2):
                            j = jj * 2 + j2
                            f = q * (FQ // P) + j
                            nc.tensor.matmul(
                                y0[:], hT[:, j, :], w2sb[f][:, 0:FQ],
                                start=(j == 0), stop=(j == FQ // P - 1),
                            )
                            nc.tensor.matmul(
                                y1[:], hT[:, j, :], w2sb[f][:, FQ:2 * FQ],
                                start=(j == 0), stop=(j == FQ // P - 1),
                            )
                    # Epilogue split into d-halves so the final-tile gate and
                    # output DMA pipeline instead of serializing.
                    for half, yp in ((0, y0), (1, y1)):
                        hsl = slice(half * FQ, (half + 1) * FQ)
                        if q == 0:
                            nc.vector.tensor_copy(ysb[t][:, hsl], yp[:])
                        else:
                            nc.vector.tensor_add(ysb[t][:, hsl], ysb[t][:, hsl], yp[:])
                        if q == NQ - 1:
                            nc.vector.tensor_scalar_mul(
                                ysb[t][:, hsl], ysb[t][:, hsl], gsb[:, t:t + 1]
                            )
                            nc.sync.dma_start(y_d[tsl, hsl], ysb[t][:, hsl])

    nc.compile()
    return nc


def _get_program(C):
    if C not in _PROGRAM_CACHE:
        _PROGRAM_CACHE[C] = _build_program(C)
    return _PROGRAM_CACHE[C]


def _install_ntff_hook():
    """Best-effort registration of the axon NTFF profile hook (trace runs)."""
    import sys, types
    try:
        from antenv.axon_hooks import get_axon_ntff_profile_hook  # noqa: F401
        return
    except ImportError:
        pass
    mod = types.ModuleType("antenv.axon_hooks")
    mod._hook = None
    mod.set_axon_ntff_profile_hook = lambda h: setattr(mod, "_hook", h)
    mod.get_axon_ntff_profile_hook = lambda: mod._hook
    sys.modules["antenv.axon_hooks"] = mod
    try:
        from trn_agent_boot.trn_boot import _ntff_profile_via_ctypes
        mod._hook = _ntff_profile_via_ctypes("/opt/axon/libaxon_pjrt.so")
    except Exception:
        pass


def _route(x, router_kernel):
    """Replicates the reference router in fp32 numpy.

    Returns (weights [T,E], order [T,2] expert ids, twn [T,2] L1-normalized
    top-2 weights)."""
    logits = x @ router_kernel  # [T, E] fp32
    m = logits.max(axis=-1, keepdims=True)
    ex = np.exp(logits - m)
    weights = ex / ex.sum(axis=-1, keepdims=True)
    order = np.argsort(-weights, axis=-1, kind="stable")[:, :TOPK]
    tw = np.take_along_axis(weights, order, axis=-1)
    twn = tw / np.abs(tw).sum(axis=-1, keepdims=True)
    return weights.astype(np.float32), order, twn.astype(np.float32)


def kernel(hidden_states, router_kernel, w1, v1, w2):
    global LAST_RESULT
    from concourse.bass_utils import run_bass_kernel_spmd

    x = np.ascontiguousarray(np.asarray(hidden_states, dtype=np.float32)).reshape(T, D)
    rk = np.asarray(router_kernel, dtype=np.float32)
    w1 = np.asarray(w1, dtype=np.float32)
    v1 = np.asarray(v1, dtype=np.float32)
    w2 = np.asarray(w2, dtype=np.float32)

    weights, order, twn = _route(x, rk)

    idxs, gates = [], []
    for e in range(E):
        mask = (order[:, 0] == e) | (order[:, 1] == e)
        idx = np.nonzero(mask)[0]
        g = np.where(order[idx, 0] == e, twn[idx, 0], twn[idx, 1])
        idxs.append(idx)
        gates.append(g.astype(np.float32))
    Cmax = max(P, -(-max(len(i) for i in idxs) // P) * P)
    # SBUF-safe capacity cap; very skewed routings run in multiple chunks.
    CAP = int(os.environ.get("KERNEL_CHUNK_CAP", "1536"))
    C = min(Cmax, CAP)
    NT = C // P
    nchunks = -(-Cmax // C)

    nc = _get_program(C)

    trace = os.environ.get("KERNEL_TRACE") == "1"
    kwargs = {}
    if trace:
        _install_ntff_hook()
        kwargs["trace"] = True
        td = os.environ.get("KERNEL_TRACE_DIR")
        if td:
            kwargs["tmpdir"] = td

    x_bf = x.astype(ml_dtypes.bfloat16)
    wmaps = []
    for e in range(E):
        wmaps.append({
            "w1t": np.ascontiguousarray(w1[e].T.astype(ml_dtypes.bfloat16)),
            "v1t": np.ascontiguousarray(v1[e].T.astype(ml_dtypes.bfloat16)),
            "w2": np.ascontiguousarray(w2[e].astype(ml_dtypes.bfloat16)),
        })

    out = np.zeros((T, D), dtype=np.float32)
    for chunk in range(nchunks):
        c0 = chunk * C
        in_maps = []
        for e in range(E):
            idx = idxs[e][c0:c0 + C]
            n = len(idx)
            xg = np.zeros((C, D), dtype=ml_dtypes.bfloat16)
            xg[:n] = x_bf[idx]
            gate_pad = np.zeros(C, dtype=np.float32)
            gate_pad[:n] = gates[e][c0:c0 + C]
            in_maps.append({
                "xT": np.ascontiguousarray(xg.T),
                "gate": np.ascontiguousarray(gate_pad.reshape(NT, P).T),
                **wmaps[e],
            })
        res = run_bass_kernel_spmd(
            nc, in_maps, core_ids=list(range(NCORES)), **kwargs
        )
        LAST_RESULT = res
        for e in range(E):
            idx = idxs[e][c0:c0 + C]
            if len(idx):
                out[idx] += res.results[e]["y"][:len(idx)]
    return out.reshape(B, S, D), weights.reshape(B, S, E)
